# revision 26
# baseline (speedup 1.0000x reference)
"""Trainium2 Bass kernel for a 2-layer LIF spiking network (T=50, B=1024,
784 -> 1024 -> 10), data-parallel over batch across 8 NeuronCores.

Strategy:
  - Layer-1 matmuls computed as cur1 = W1 @ x[t].T so hidden lands on
    partitions. fp32 accuracy via an unscaled 3-term fp16 emulation,
    densely packed on the contraction axis: moving rows
    [xh; (x-xh)*2048; xh] (2352 rows -> 19 k-tiles of 128) against
    stationary [Wh; Wh/2048; Wl].  End-to-end error ~5e-7 relative.
  - Software-pipelined issue order: PE stream is mm1(c0), mm1(c1),
    mm2(c0), mm1(c2), mm2(c1), ... so the tensor engine never waits on
    the LIF chain (keeps the 2.4 GHz p-state).  Chunk sizes taper at the
    ends ([1,4,...,4,2,1,1,1]) so the vector LIF chain drains under the
    shrinking matmul stream.
  - LIF updates are 3 fused DVE ops per step (u = b*m + cur;
    m = (spk_prev==0)*u; spk = m > 1 -- the previous spike doubles as
    the zero-reset mask).  Layer-2 LIF uses the rotating output buffer
    as its state (no extra state-copy ops).
"""

import sys

import numpy as np

sys.path.insert(0, "/opt/trn_rl_repo")

T, B, N_IN, N_HID, N_OUT = 50, 1024, 784, 1024, 10
NCORES = 8
BS = B // NCORES            # batch shard per core = 128
HT = N_HID // 128           # 8 hidden tiles
KROWS = 3 * N_IN            # 2352 packed contraction rows
KT2 = (KROWS + 127) // 128  # 19 k-tiles
CHUNK = 4                   # timesteps per chunk (N = 512)
TB = T * BS                 # 6400

LAST_RESULT = None          # BassKernelResults of the last run (for test.py)


def _build_bass(b1: float, b2: float):
    import concourse.bass as bass
    from concourse import bacc
    import concourse.mybir as mybir
    import concourse.tile as tile

    f32 = mybir.dt.float32
    f16 = mybir.dt.float16
    Alu = mybir.AluOpType
    Act = mybir.ActivationFunctionType

    nc = bacc.Bacc("TRN2", target_bir_lowering=False, debug=False,
                   num_devices=NCORES)

    xm_d = nc.dram_tensor("xm", [128, KT2, TB], f16, kind="ExternalInput")
    wm_d = nc.dram_tensor("wm", [128, HT, KT2 * 128], f16, kind="ExternalInput")
    w2h_d = nc.dram_tensor("w2h", [128, HT * N_OUT], f16, kind="ExternalInput")
    w2l_d = nc.dram_tensor("w2l", [128, HT * N_OUT], f16, kind="ExternalInput")
    spk_d = nc.dram_tensor("spk2o", [N_OUT, TB], f32, kind="ExternalOutput")
    mem_d = nc.dram_tensor("mem2o", [N_OUT, TB], f32, kind="ExternalOutput")

    # tiny first chunk (fast start: less DMA before the first matmul) and a
    # tapered tail: the vector LIF chain (~3.6us/step) must drain under the
    # shrinking mm1 stream, else the PE idles at the end waiting on spikes
    sizes = [1] + [CHUNK] * 11 + [2, 1, 1, 1]
    assert sum(sizes) == T
    chunks = []
    c0 = 0
    for s in sizes:
        chunks.append((c0, s))
        c0 += s
    NCH = len(chunks)

    with tile.TileContext(nc) as tc:
        with (
            tc.tile_pool(name="const", bufs=1) as cpool,
            tc.tile_pool(name="xs", bufs=2) as xpool,
            tc.tile_pool(name="cur", bufs=2) as curpool,
            tc.tile_pool(name="spk", bufs=2) as spkpool,
            tc.tile_pool(name="state", bufs=1) as stpool,
            tc.tile_pool(name="outst", bufs=2) as opool,
            tc.tile_pool(name="ps1", bufs=3, space="PSUM") as ps1pool,
            tc.tile_pool(name="ps2", bufs=2, space="PSUM") as ps2pool,
        ):
            wm = cpool.tile([128, HT, KT2, 128], f16)
            w2h = cpool.tile([128, HT, N_OUT], f16)
            w2l = cpool.tile([128, HT, N_OUT], f16)

            # persistent LIF state
            m1 = stpool.tile([128, HT, 128], f32)
            u1 = stpool.tile([128, HT, 128], f32)
            u2 = stpool.tile([N_OUT, 128], f32)
            mem_init = stpool.tile([N_OUT, 128], f32)
            spk1_init = stpool.tile([128, HT, 128], f16)
            spk2_init = stpool.tile([N_OUT, 128], f32)
            nc.vector.memset(m1[:], 0.0)
            nc.vector.memset(mem_init[:], 0.0)
            nc.vector.memset(spk1_init[:], 0.0)
            nc.vector.memset(spk2_init[:], 0.0)

            xw = {}

            def issue_xdma(ci):
                c0_, csz_ = chunks[ci]
                nw = csz_ * BS
                t = xpool.tile([128, KT2, nw], f16, tag="xw", name=f"xw{ci}")
                win = slice(c0_ * BS, c0_ * BS + nw)
                nc.gpsimd.dma_start(t[:, 0:10, :], xm_d[:, 0:10, win])
                nc.sync.dma_start(t[:, 10:KT2, :], xm_d[:, 10:KT2, win])
                xw[ci] = t

            issue_xdma(0)
            # first weight tile split across both rings so the very first
            # matmul's operands land as early as possible
            nc.gpsimd.dma_start(wm[:, 0, 0:10, :], wm_d[:, 0, 0:10 * 128])
            nc.sync.dma_start(wm[:, 0, 10:KT2, :],
                              wm_d[:, 0, 10 * 128:KT2 * 128])
            for h in range(1, HT):
                nc.sync.dma_start(wm[:, h], wm_d[:, h])
            nc.sync.dma_start(w2h[:], w2h_d[:])
            nc.sync.dma_start(w2l[:], w2l_d[:])

            mem_prev = [mem_init]
            spk1_prev = [spk1_init[:]]
            spk2_prev = [spk2_init[:]]

            def tail(ci, cur, spk):
                c0_, csz_ = chunks[ci]
                nw = csz_ * BS
                # ---- LIF1: 3 fused ops per step
                # u = b1*m + cur ; m' = (spk_prev == 0) * u  (zero-reset from
                # the previous step's spike == previous mem > thresh) ;
                # spk = m' > thresh
                for j in range(csz_):
                    cs = slice(j * BS, (j + 1) * BS)
                    nc.vector.scalar_tensor_tensor(
                        u1[:], m1[:], b1, cur[:, :, cs],
                        op0=Alu.mult, op1=Alu.add)
                    nc.vector.scalar_tensor_tensor(
                        m1[:], spk1_prev[0], 0.0, u1[:],
                        op0=Alu.is_equal, op1=Alu.mult)
                    nc.vector.tensor_scalar(
                        spk[:, :, cs], m1[:], 1.0, None, op0=Alu.is_gt)
                    spk1_prev[0] = spk[:, :, cs]
                # ---- layer 2: col-packed 4 groups x (2h x hi/lo)
                p2 = ps2pool.tile([128, nw], f32, tag="p2", name=f"p2_{ci}")
                for cg in range(4):
                    po = 32 * cg
                    ip = 0
                    for h in (2 * cg, 2 * cg + 1):
                        for wsb in (w2h, w2l):
                            nc.tensor.matmul(
                                p2[po:po + N_OUT, :], wsb[:, h, :],
                                spk[:, h, :],
                                start=(ip == 0), stop=(ip == 3),
                                tile_position=(0, po))
                            ip += 1
                c2 = opool.tile([N_OUT, nw], f32, tag="c2", name=f"c2_{ci}")
                nc.scalar.activation(c2[:], p2[0:N_OUT, :], Act.Copy)
                for cg in (1, 2, 3):
                    po = 32 * cg
                    nc.vector.scalar_tensor_tensor(
                        c2[:], p2[po:po + N_OUT, :], 1.0, c2[:],
                        op0=Alu.bypass, op1=Alu.add)
                # ---- LIF2: rotating mem_st doubles as state
                spk_st = opool.tile([N_OUT, nw], f32, tag="spkst",
                                    name=f"spkst{ci}")
                mem_st = opool.tile([N_OUT, nw], f32, tag="memst",
                                    name=f"memst{ci}")
                for j in range(csz_):
                    cs = slice(j * BS, (j + 1) * BS)
                    mp = mem_prev[0] if j == 0 else mem_st[:, (j - 1) * BS:j * BS]
                    nc.vector.scalar_tensor_tensor(
                        u2[:], mp, b2, c2[:, cs], op0=Alu.mult, op1=Alu.add)
                    nc.vector.scalar_tensor_tensor(
                        mem_st[:, cs], spk2_prev[0], 0.0, u2[:],
                        op0=Alu.is_equal, op1=Alu.mult)
                    nc.vector.tensor_scalar(
                        spk_st[:, cs], mem_st[:, cs], 1.0, None, op0=Alu.is_gt)
                    spk2_prev[0] = spk_st[:, cs]
                mem_prev[0] = mem_st[:, (csz_ - 1) * BS:csz_ * BS]
                ow = slice(c0_ * BS, (c0_ + csz_) * BS)
                nc.gpsimd.dma_start(spk_d[:, ow], spk_st[:])
                nc.gpsimd.dma_start(mem_d[:, ow], mem_st[:])

            prev = None
            for ci, (c0_, csz_) in enumerate(chunks):
                nw = csz_ * BS
                if ci + 1 < NCH:
                    issue_xdma(ci + 1)
                cur = curpool.tile([128, HT, nw], f32, tag="cur",
                                   name=f"cur{ci}")
                spk = spkpool.tile([128, HT, nw], f16, tag="spk",
                                   name=f"spk{ci}")
                for h in range(HT):
                    ps = ps1pool.tile([128, nw], f32, tag="p1",
                                      name=f"p1_{ci}_{h}")
                    for j in range(KT2):
                        # rows 2352:2432 are zero-padded on host; full-128
                        # tiles keep walrus codegen on the well-trodden path
                        nc.tensor.matmul(
                            ps[:], wm[:, h, j, :], xw[ci][:, j, :],
                            start=(j == 0), stop=(j == KT2 - 1))
                    nc.scalar.activation(cur[:, h, :], ps[:], Act.Copy)
                if prev is not None:
                    tail(*prev)
                prev = (ci, cur, spk)
            tail(*prev)

    nc.compile()
    return nc


def _prep_inputs(x, W1, W2):
    """Host-side layout + fp16 3-term splits."""
    f32 = np.float32
    f16 = np.float16
    SH = f32(2048.0)

    # x: [T, B, N_IN] -> feature-major [N_IN, T, B]
    xt = np.ascontiguousarray(np.transpose(np.asarray(x, f32), (2, 0, 1)))
    xh = xt.astype(f16)
    xl = ((xt - xh.astype(f32)) * SH).astype(f16)

    xcores = []
    for c in range(NCORES):
        bs = slice(c * BS, (c + 1) * BS)
        xc = np.zeros((KT2 * 128, T * BS), dtype=f16)
        xc[0:N_IN] = xh[:, :, bs].reshape(N_IN, T * BS)
        xc[N_IN:2 * N_IN] = xl[:, :, bs].reshape(N_IN, T * BS)
        xc[2 * N_IN:3 * N_IN] = xc[0:N_IN]
        # [KROWSpad, TB] -> [128, KT2, TB]
        xc = np.ascontiguousarray(
            xc.reshape(KT2, 128, T * BS).transpose(1, 0, 2))
        xcores.append(xc)

    W1T = np.ascontiguousarray(np.asarray(W1, f32).T)   # [784, 1024]
    wh = W1T.astype(f16)
    wl = (W1T - wh.astype(f32)).astype(f16)
    whs = (wh.astype(f32) / SH).astype(f16)
    wcat = np.zeros((KT2 * 128, N_HID), dtype=f16)
    wcat[0:N_IN] = wh
    wcat[N_IN:2 * N_IN] = whs
    wcat[2 * N_IN:3 * N_IN] = wl
    # [KROWSpad, 1024] -> [128, HT, KT2, 128] with free=(h, j, m)
    wm = np.ascontiguousarray(
        wcat.reshape(KT2, 128, HT, 128).transpose(1, 2, 0, 3).reshape(
            128, HT, KT2 * 128))

    W2T = np.ascontiguousarray(np.asarray(W2, f32).T)   # [1024, 10]
    w2h = W2T.astype(f16)
    w2l = (W2T - w2h.astype(f32)).astype(f16)

    def w2_layout(a):
        return np.ascontiguousarray(
            a.reshape(HT, 128, N_OUT).transpose(1, 0, 2).reshape(
                128, HT * N_OUT))

    weights = {"wm": wm, "w2h": w2_layout(w2h), "w2l": w2_layout(w2l)}
    return xcores, weights


def _ensure_ntff_shim():
    """run_bass_kernel_spmd(trace) imports antenv.axon_hooks, absent in some
    images; install a graceful stand-in so tracing degrades instead of
    crashing."""
    try:
        import antenv.axon_hooks  # noqa: F401
        return
    except Exception:
        pass
    import types
    hook = None
    try:
        from trn_agent_boot.trn_boot import _ntff_profile_via_ctypes
        hook = _ntff_profile_via_ctypes("/opt/axon/libaxon_pjrt.so")
    except Exception:
        hook = None
    mod = types.ModuleType("antenv.axon_hooks")
    mod._hook = hook
    mod.get_axon_ntff_profile_hook = lambda: mod._hook
    mod.set_axon_ntff_profile_hook = lambda h: setattr(mod, "_hook", h)
    sys.modules["antenv.axon_hooks"] = mod


def kernel(x, W1, W2, beta1, beta2):
    global LAST_RESULT
    from concourse.bass_utils import run_bass_kernel_spmd

    _ensure_ntff_shim()

    b1 = float(np.clip(np.float32(beta1), 0.0, 1.0))
    b2 = float(np.clip(np.float32(beta2), 0.0, 1.0))

    xcores, weights = _prep_inputs(x, W1, W2)
    nc = _build_bass(b1, b2)

    in_maps = []
    for c in range(NCORES):
        m = {"xm": xcores[c]}
        m.update(weights)
        in_maps.append(m)

    res = run_bass_kernel_spmd(nc, in_maps, core_ids=list(range(NCORES)))
    LAST_RESULT = res

    spk_parts, mem_parts = [], []
    for c in range(NCORES):
        r = res.results[c]
        spk_parts.append(
            r["spk2o"].reshape(N_OUT, T, BS).transpose(1, 2, 0))
        mem_parts.append(
            r["mem2o"].reshape(N_OUT, T, BS).transpose(1, 2, 0))
    spk2 = np.ascontiguousarray(np.concatenate(spk_parts, axis=1))
    mem2 = np.ascontiguousarray(np.concatenate(mem_parts, axis=1))
    return spk2, mem2


# revision 28
# speedup vs baseline: 1.1913x; 1.1913x over previous
"""Trainium2 Bass kernel for a 2-layer LIF spiking network (T=50, B=1024,
784 -> 1024 -> 10), data-parallel over batch across 8 NeuronCores.

Strategy:
  - Layer-1 matmuls computed as cur1 = W1 @ x[t].T so hidden lands on
    partitions. fp32 accuracy via an unscaled 3-term fp16 emulation,
    densely packed on the contraction axis: moving rows
    [xh; (x-xh)*2048; xh] (2352 rows -> 19 k-tiles of 128) against
    stationary [Wh; Wh/2048; Wl].  End-to-end error ~5e-7 relative.
  - Software-pipelined issue order: PE stream is mm1(c0), mm1(c1),
    mm2(c0), mm1(c2), mm2(c1), ... so the tensor engine never waits on
    the LIF chain (keeps the 2.4 GHz p-state).  Chunk sizes taper at the
    ends ([1,4,...,4,2,1,1,1]) so the vector LIF chain drains under the
    shrinking matmul stream.
  - LIF updates are 3 fused DVE ops per step (u = b*m + cur;
    m = (spk_prev==0)*u; spk = m > 1 -- the previous spike doubles as
    the zero-reset mask).  Layer-2 LIF uses the rotating output buffer
    as its state (no extra state-copy ops).
"""

import sys

import numpy as np

sys.path.insert(0, "/opt/trn_rl_repo")

T, B, N_IN, N_HID, N_OUT = 50, 1024, 784, 1024, 10
NCORES = 8
BS = B // NCORES            # batch shard per core = 128
HT = N_HID // 128           # 8 hidden tiles
KROWS = 3 * N_IN            # 2352 packed contraction rows
KT2 = (KROWS + 127) // 128  # 19 k-tiles
CHUNK = 4                   # timesteps per chunk (N = 512)
TB = T * BS                 # 6400

LAST_RESULT = None          # BassKernelResults of the last run (for test.py)


def _build_bass(b1: float, b2: float):
    import concourse.bass as bass
    from concourse import bacc
    import concourse.mybir as mybir
    import concourse.tile as tile

    f32 = mybir.dt.float32
    f16 = mybir.dt.float16
    Alu = mybir.AluOpType
    Act = mybir.ActivationFunctionType

    nc = bacc.Bacc("TRN2", target_bir_lowering=False, debug=False,
                   num_devices=NCORES)

    xm_d = nc.dram_tensor("xm", [128, KT2, TB], f16, kind="ExternalInput")
    wm_d = nc.dram_tensor("wm", [128, HT, KT2 * 128], f16, kind="ExternalInput")
    w2h_d = nc.dram_tensor("w2h", [128, HT * N_OUT], f16, kind="ExternalInput")
    w2l_d = nc.dram_tensor("w2l", [128, HT * N_OUT], f16, kind="ExternalInput")
    spk_d = nc.dram_tensor("spk2o", [N_OUT, TB], f32, kind="ExternalOutput")
    mem_d = nc.dram_tensor("mem2o", [N_OUT, TB], f32, kind="ExternalOutput")

    # tiny first chunk (fast start: less DMA before the first matmul) and a
    # tapered tail: the vector LIF chain (~3.6us/step) must drain under the
    # shrinking mm1 stream, else the PE idles at the end waiting on spikes
    sizes = [1] + [CHUNK] * 11 + [2, 1, 1, 1]
    assert sum(sizes) == T
    chunks = []
    c0 = 0
    for s in sizes:
        chunks.append((c0, s))
        c0 += s
    NCH = len(chunks)

    with tile.TileContext(nc) as tc:
        with (
            tc.tile_pool(name="const", bufs=1) as cpool,
            tc.tile_pool(name="xs", bufs=2) as xpool,
            tc.tile_pool(name="cur", bufs=2) as curpool,
            tc.tile_pool(name="spk", bufs=2) as spkpool,
            tc.tile_pool(name="state", bufs=1) as stpool,
            tc.tile_pool(name="outst", bufs=2) as opool,
            tc.tile_pool(name="ps1", bufs=3, space="PSUM") as ps1pool,
            tc.tile_pool(name="ps2", bufs=2, space="PSUM") as ps2pool,
        ):
            wm = cpool.tile([128, HT, KT2, 128], f16)
            w2h = cpool.tile([128, HT, N_OUT], f16)
            w2l = cpool.tile([128, HT, N_OUT], f16)

            # persistent LIF state
            m1 = stpool.tile([128, HT, 128], f32)
            u1 = stpool.tile([128, HT, 128], f32)
            u2 = stpool.tile([N_OUT, 128], f32)
            mem_init = stpool.tile([N_OUT, 128], f32)
            spk1_init = stpool.tile([128, HT, 128], f16)
            spk2_init = stpool.tile([N_OUT, 128], f32)
            nc.vector.memset(m1[:], 0.0)
            nc.vector.memset(mem_init[:], 0.0)
            nc.vector.memset(spk1_init[:], 0.0)
            nc.vector.memset(spk2_init[:], 0.0)

            xw = {}

            def issue_xdma(ci):
                # two tiles per window (one per DMA ring) so the first
                # matmuls only wait on the ring that carries k-tiles 0:10
                c0_, csz_ = chunks[ci]
                nw = csz_ * BS
                ta = xpool.tile([128, 10, nw], f16, tag="xwa", name=f"xwa{ci}")
                tb = xpool.tile([128, KT2 - 10, nw], f16, tag="xwb",
                                name=f"xwb{ci}")
                win = slice(c0_ * BS, c0_ * BS + nw)
                nc.gpsimd.dma_start(ta[:], xm_d[:, 0:10, win])
                nc.sync.dma_start(tb[:], xm_d[:, 10:KT2, win])
                xw[ci] = (ta, tb)

            issue_xdma(0)
            # first weight tile split across both rings so the very first
            # matmul's operands land as early as possible
            nc.gpsimd.dma_start(wm[:, 0, 0:10, :], wm_d[:, 0, 0:10 * 128])
            nc.sync.dma_start(wm[:, 0, 10:KT2, :],
                              wm_d[:, 0, 10 * 128:KT2 * 128])
            for h in range(1, HT):
                nc.sync.dma_start(wm[:, h], wm_d[:, h])
            nc.sync.dma_start(w2h[:], w2h_d[:])
            nc.sync.dma_start(w2l[:], w2l_d[:])

            mem_prev = [mem_init]
            spk1_prev = [spk1_init[:]]
            spk2_prev = [spk2_init[:]]

            def tail(ci, cur, spk):
                c0_, csz_ = chunks[ci]
                nw = csz_ * BS
                # ---- LIF1: 3 fused ops per step
                # u = b1*m + cur ; m' = (spk_prev == 0) * u  (zero-reset from
                # the previous step's spike == previous mem > thresh) ;
                # spk = m' > thresh
                for j in range(csz_):
                    cs = slice(j * BS, (j + 1) * BS)
                    nc.vector.scalar_tensor_tensor(
                        u1[:], m1[:], b1, cur[:, :, cs],
                        op0=Alu.mult, op1=Alu.add)
                    nc.vector.scalar_tensor_tensor(
                        m1[:], spk1_prev[0], 0.0, u1[:],
                        op0=Alu.is_equal, op1=Alu.mult)
                    nc.vector.tensor_scalar(
                        spk[:, :, cs], m1[:], 1.0, None, op0=Alu.is_gt)
                    spk1_prev[0] = spk[:, :, cs]
                # ---- layer 2: col-packed 4 groups x (2h x hi/lo)
                p2 = ps2pool.tile([128, nw], f32, tag="p2", name=f"p2_{ci}")
                for cg in range(4):
                    po = 32 * cg
                    ip = 0
                    for h in (2 * cg, 2 * cg + 1):
                        for wsb in (w2h, w2l):
                            nc.tensor.matmul(
                                p2[po:po + N_OUT, :], wsb[:, h, :],
                                spk[:, h, :],
                                start=(ip == 0), stop=(ip == 3),
                                tile_position=(0, po))
                            ip += 1
                c2 = opool.tile([N_OUT, nw], f32, tag="c2", name=f"c2_{ci}")
                nc.scalar.activation(c2[:], p2[0:N_OUT, :], Act.Copy)
                for cg in (1, 2, 3):
                    po = 32 * cg
                    nc.vector.scalar_tensor_tensor(
                        c2[:], p2[po:po + N_OUT, :], 1.0, c2[:],
                        op0=Alu.bypass, op1=Alu.add)
                # ---- LIF2: rotating mem_st doubles as state
                spk_st = opool.tile([N_OUT, nw], f32, tag="spkst",
                                    name=f"spkst{ci}")
                mem_st = opool.tile([N_OUT, nw], f32, tag="memst",
                                    name=f"memst{ci}")
                for j in range(csz_):
                    cs = slice(j * BS, (j + 1) * BS)
                    mp = mem_prev[0] if j == 0 else mem_st[:, (j - 1) * BS:j * BS]
                    nc.vector.scalar_tensor_tensor(
                        u2[:], mp, b2, c2[:, cs], op0=Alu.mult, op1=Alu.add)
                    nc.vector.scalar_tensor_tensor(
                        mem_st[:, cs], spk2_prev[0], 0.0, u2[:],
                        op0=Alu.is_equal, op1=Alu.mult)
                    nc.vector.tensor_scalar(
                        spk_st[:, cs], mem_st[:, cs], 1.0, None, op0=Alu.is_gt)
                    spk2_prev[0] = spk_st[:, cs]
                mem_prev[0] = mem_st[:, (csz_ - 1) * BS:csz_ * BS]
                ow = slice(c0_ * BS, (c0_ + csz_) * BS)
                nc.gpsimd.dma_start(spk_d[:, ow], spk_st[:])
                nc.gpsimd.dma_start(mem_d[:, ow], mem_st[:])

            prev = None
            for ci, (c0_, csz_) in enumerate(chunks):
                nw = csz_ * BS
                if ci + 1 < NCH:
                    issue_xdma(ci + 1)
                cur = curpool.tile([128, HT, nw], f32, tag="cur",
                                   name=f"cur{ci}")
                spk = spkpool.tile([128, HT, nw], f16, tag="spk",
                                   name=f"spk{ci}")
                for h in range(HT):
                    ps = ps1pool.tile([128, nw], f32, tag="p1",
                                      name=f"p1_{ci}_{h}")
                    for j in range(KT2):
                        # rows 2352:2432 are zero-padded on host; full-128
                        # tiles keep walrus codegen on the well-trodden path
                        xa, xb = xw[ci]
                        rhs = xa[:, j, :] if j < 10 else xb[:, j - 10, :]
                        nc.tensor.matmul(
                            ps[:], wm[:, h, j, :], rhs,
                            start=(j == 0), stop=(j == KT2 - 1))
                    nc.scalar.activation(cur[:, h, :], ps[:], Act.Copy)
                if prev is not None:
                    tail(*prev)
                prev = (ci, cur, spk)
            tail(*prev)

    nc.compile()
    return nc


def _prep_inputs(x, W1, W2):
    """Host-side layout + fp16 3-term splits."""
    f32 = np.float32
    f16 = np.float16
    SH = f32(2048.0)

    # x: [T, B, N_IN] -> feature-major [N_IN, T, B]
    xt = np.ascontiguousarray(np.transpose(np.asarray(x, f32), (2, 0, 1)))
    xh = xt.astype(f16)
    xl = ((xt - xh.astype(f32)) * SH).astype(f16)

    xcores = []
    for c in range(NCORES):
        bs = slice(c * BS, (c + 1) * BS)
        xc = np.zeros((KT2 * 128, T * BS), dtype=f16)
        xc[0:N_IN] = xh[:, :, bs].reshape(N_IN, T * BS)
        xc[N_IN:2 * N_IN] = xl[:, :, bs].reshape(N_IN, T * BS)
        xc[2 * N_IN:3 * N_IN] = xc[0:N_IN]
        # [KROWSpad, TB] -> [128, KT2, TB]
        xc = np.ascontiguousarray(
            xc.reshape(KT2, 128, T * BS).transpose(1, 0, 2))
        xcores.append(xc)

    W1T = np.ascontiguousarray(np.asarray(W1, f32).T)   # [784, 1024]
    wh = W1T.astype(f16)
    wl = (W1T - wh.astype(f32)).astype(f16)
    whs = (wh.astype(f32) / SH).astype(f16)
    wcat = np.zeros((KT2 * 128, N_HID), dtype=f16)
    wcat[0:N_IN] = wh
    wcat[N_IN:2 * N_IN] = whs
    wcat[2 * N_IN:3 * N_IN] = wl
    # [KROWSpad, 1024] -> [128, HT, KT2, 128] with free=(h, j, m)
    wm = np.ascontiguousarray(
        wcat.reshape(KT2, 128, HT, 128).transpose(1, 2, 0, 3).reshape(
            128, HT, KT2 * 128))

    W2T = np.ascontiguousarray(np.asarray(W2, f32).T)   # [1024, 10]
    w2h = W2T.astype(f16)
    w2l = (W2T - w2h.astype(f32)).astype(f16)

    def w2_layout(a):
        return np.ascontiguousarray(
            a.reshape(HT, 128, N_OUT).transpose(1, 0, 2).reshape(
                128, HT * N_OUT))

    weights = {"wm": wm, "w2h": w2_layout(w2h), "w2l": w2_layout(w2l)}
    return xcores, weights


def _ensure_ntff_shim():
    """run_bass_kernel_spmd(trace) imports antenv.axon_hooks, absent in some
    images; install a graceful stand-in so tracing degrades instead of
    crashing."""
    try:
        import antenv.axon_hooks  # noqa: F401
        return
    except Exception:
        pass
    import types
    hook = None
    try:
        from trn_agent_boot.trn_boot import _ntff_profile_via_ctypes
        hook = _ntff_profile_via_ctypes("/opt/axon/libaxon_pjrt.so")
    except Exception:
        hook = None
    mod = types.ModuleType("antenv.axon_hooks")
    mod._hook = hook
    mod.get_axon_ntff_profile_hook = lambda: mod._hook
    mod.set_axon_ntff_profile_hook = lambda h: setattr(mod, "_hook", h)
    sys.modules["antenv.axon_hooks"] = mod


def kernel(x, W1, W2, beta1, beta2):
    global LAST_RESULT
    from concourse.bass_utils import run_bass_kernel_spmd

    _ensure_ntff_shim()

    b1 = float(np.clip(np.float32(beta1), 0.0, 1.0))
    b2 = float(np.clip(np.float32(beta2), 0.0, 1.0))

    xcores, weights = _prep_inputs(x, W1, W2)
    nc = _build_bass(b1, b2)

    in_maps = []
    for c in range(NCORES):
        m = {"xm": xcores[c]}
        m.update(weights)
        in_maps.append(m)

    res = run_bass_kernel_spmd(nc, in_maps, core_ids=list(range(NCORES)))
    LAST_RESULT = res

    spk_parts, mem_parts = [], []
    for c in range(NCORES):
        r = res.results[c]
        spk_parts.append(
            r["spk2o"].reshape(N_OUT, T, BS).transpose(1, 2, 0))
        mem_parts.append(
            r["mem2o"].reshape(N_OUT, T, BS).transpose(1, 2, 0))
    spk2 = np.ascontiguousarray(np.concatenate(spk_parts, axis=1))
    mem2 = np.ascontiguousarray(np.concatenate(mem_parts, axis=1))
    return spk2, mem2


# revision 29
# speedup vs baseline: 1.1919x; 1.0005x over previous
"""Trainium2 Bass kernel for a 2-layer LIF spiking network (T=50, B=1024,
784 -> 1024 -> 10), data-parallel over batch across 8 NeuronCores.

Strategy:
  - Layer-1 matmuls computed as cur1 = W1 @ x[t].T so hidden lands on
    partitions. fp32 accuracy via an unscaled 3-term fp16 emulation,
    densely packed on the contraction axis: moving rows
    [xh; (x-xh)*2048; xh] (2352 rows -> 19 k-tiles of 128) against
    stationary [Wh; Wh/2048; Wl].  End-to-end error ~5e-7 relative.
  - Software-pipelined issue order: PE stream is mm1(c0), mm1(c1),
    mm2(c0), mm1(c2), mm2(c1), ... so the tensor engine never waits on
    the LIF chain (keeps the 2.4 GHz p-state).  Chunk sizes taper at the
    ends ([1,4,...,4,2,1,1,1]) so the vector LIF chain drains under the
    shrinking matmul stream.
  - LIF updates are 3 fused DVE ops per step (u = b*m + cur;
    m = (spk_prev==0)*u; spk = m > 1 -- the previous spike doubles as
    the zero-reset mask).  Layer-2 LIF uses the rotating output buffer
    as its state (no extra state-copy ops).
"""

import sys

import numpy as np

sys.path.insert(0, "/opt/trn_rl_repo")

T, B, N_IN, N_HID, N_OUT = 50, 1024, 784, 1024, 10
NCORES = 8
BS = B // NCORES            # batch shard per core = 128
HT = N_HID // 128           # 8 hidden tiles
KROWS = 3 * N_IN            # 2352 packed contraction rows
KT2 = (KROWS + 127) // 128  # 19 k-tiles
CHUNK = 4                   # timesteps per chunk (N = 512)
TB = T * BS                 # 6400

LAST_RESULT = None          # BassKernelResults of the last run (for test.py)


def _build_bass(b1: float, b2: float):
    import concourse.bass as bass
    from concourse import bacc
    import concourse.mybir as mybir
    import concourse.tile as tile

    f32 = mybir.dt.float32
    f16 = mybir.dt.float16
    Alu = mybir.AluOpType
    Act = mybir.ActivationFunctionType

    nc = bacc.Bacc("TRN2", target_bir_lowering=False, debug=False,
                   num_devices=NCORES)

    xm_d = nc.dram_tensor("xm", [128, KT2, TB], f16, kind="ExternalInput")
    wm_d = nc.dram_tensor("wm", [128, HT, KT2 * 128], f16, kind="ExternalInput")
    w2h_d = nc.dram_tensor("w2h", [128, HT * N_OUT], f16, kind="ExternalInput")
    w2l_d = nc.dram_tensor("w2l", [128, HT * N_OUT], f16, kind="ExternalInput")
    spk_d = nc.dram_tensor("spk2o", [N_OUT, TB], f32, kind="ExternalOutput")
    mem_d = nc.dram_tensor("mem2o", [N_OUT, TB], f32, kind="ExternalOutput")

    # tiny first chunk (fast start: less DMA before the first matmul) and a
    # tapered tail: the vector LIF chain (~3.6us/step) must drain under the
    # shrinking mm1 stream, else the PE idles at the end waiting on spikes
    sizes = [1] + [CHUNK] * 11 + [2, 1, 1, 1]
    assert sum(sizes) == T
    chunks = []
    c0 = 0
    for s in sizes:
        chunks.append((c0, s))
        c0 += s
    NCH = len(chunks)

    with tile.TileContext(nc) as tc:
        with (
            tc.tile_pool(name="const", bufs=1) as cpool,
            tc.tile_pool(name="xs", bufs=2) as xpool,
            tc.tile_pool(name="cur", bufs=2) as curpool,
            tc.tile_pool(name="spk", bufs=2) as spkpool,
            tc.tile_pool(name="state", bufs=1) as stpool,
            tc.tile_pool(name="outst", bufs=2) as opool,
            tc.tile_pool(name="ps1", bufs=3, space="PSUM") as ps1pool,
            tc.tile_pool(name="ps2", bufs=2, space="PSUM") as ps2pool,
        ):
            wm = cpool.tile([128, HT, KT2, 128], f16)
            w2h = cpool.tile([128, HT, N_OUT], f16)
            w2l = cpool.tile([128, HT, N_OUT], f16)

            # persistent LIF state
            m1 = stpool.tile([128, HT, 128], f32)
            u1 = stpool.tile([128, HT, 128], f32)
            u2 = stpool.tile([N_OUT, 128], f32)
            mem_init = stpool.tile([N_OUT, 128], f32)
            spk1_init = stpool.tile([128, HT, 128], f16)
            spk2_init = stpool.tile([N_OUT, 128], f32)
            nc.vector.memset(m1[:], 0.0)
            nc.vector.memset(mem_init[:], 0.0)
            nc.vector.memset(spk1_init[:], 0.0)
            nc.vector.memset(spk2_init[:], 0.0)

            xw = {}

            def issue_xdma(ci):
                # two tiles per window (one per DMA ring) so the first
                # matmuls only wait on the ring that carries k-tiles 0:10
                c0_, csz_ = chunks[ci]
                nw = csz_ * BS
                ta = xpool.tile([128, 10, nw], f16, tag="xwa", name=f"xwa{ci}")
                tb = xpool.tile([128, KT2 - 10, nw], f16, tag="xwb",
                                name=f"xwb{ci}")
                win = slice(c0_ * BS, c0_ * BS + nw)
                nc.gpsimd.dma_start(ta[:], xm_d[:, 0:10, win])
                nc.sync.dma_start(tb[:], xm_d[:, 10:KT2, win])
                xw[ci] = (ta, tb)

            issue_xdma(0)
            # first weight tile split across both rings so the very first
            # matmul's operands land as early as possible
            nc.gpsimd.dma_start(wm[:, 0, 0:10, :], wm_d[:, 0, 0:10 * 128])
            nc.sync.dma_start(wm[:, 0, 10:KT2, :],
                              wm_d[:, 0, 10 * 128:KT2 * 128])
            for h in range(1, HT):
                nc.sync.dma_start(wm[:, h], wm_d[:, h])
            nc.sync.dma_start(w2h[:], w2h_d[:])
            nc.sync.dma_start(w2l[:], w2l_d[:])

            mem_prev = [mem_init]
            spk1_prev = [spk1_init[:]]
            spk2_prev = [spk2_init[:]]

            def tail(ci, cur, spk):
                c0_, csz_ = chunks[ci]
                nw = csz_ * BS
                # ---- LIF1: 3 fused ops per step
                # u = b1*m + cur ; m' = (spk_prev == 0) * u  (zero-reset from
                # the previous step's spike == previous mem > thresh) ;
                # spk = m' > thresh
                for j in range(csz_):
                    cs = slice(j * BS, (j + 1) * BS)
                    nc.vector.scalar_tensor_tensor(
                        u1[:], m1[:], b1, cur[:, :, cs],
                        op0=Alu.mult, op1=Alu.add)
                    nc.vector.scalar_tensor_tensor(
                        m1[:], spk1_prev[0], 0.0, u1[:],
                        op0=Alu.is_equal, op1=Alu.mult)
                    nc.vector.tensor_scalar(
                        spk[:, :, cs], m1[:], 1.0, None, op0=Alu.is_gt)
                    spk1_prev[0] = spk[:, :, cs]
                # ---- layer 2: col-packed 4 groups x (2h x hi/lo),
                # pass-major issue so all four 32-col strips run concurrently
                p2 = ps2pool.tile([128, nw], f32, tag="p2", name=f"p2_{ci}")
                for ip in range(4):
                    for cg in range(4):
                        po = 32 * cg
                        h = 2 * cg + ip // 2
                        wsb = (w2h, w2l)[ip % 2]
                        nc.tensor.matmul(
                            p2[po:po + N_OUT, :], wsb[:, h, :],
                            spk[:, h, :],
                            start=(ip == 0), stop=(ip == 3),
                            tile_position=(0, po))
                c2 = opool.tile([N_OUT, nw], f32, tag="c2", name=f"c2_{ci}")
                nc.scalar.activation(c2[:], p2[0:N_OUT, :], Act.Copy)
                for cg in (1, 2, 3):
                    po = 32 * cg
                    nc.vector.scalar_tensor_tensor(
                        c2[:], p2[po:po + N_OUT, :], 1.0, c2[:],
                        op0=Alu.bypass, op1=Alu.add)
                # ---- LIF2: rotating mem_st doubles as state
                spk_st = opool.tile([N_OUT, nw], f32, tag="spkst",
                                    name=f"spkst{ci}")
                mem_st = opool.tile([N_OUT, nw], f32, tag="memst",
                                    name=f"memst{ci}")
                for j in range(csz_):
                    cs = slice(j * BS, (j + 1) * BS)
                    mp = mem_prev[0] if j == 0 else mem_st[:, (j - 1) * BS:j * BS]
                    nc.vector.scalar_tensor_tensor(
                        u2[:], mp, b2, c2[:, cs], op0=Alu.mult, op1=Alu.add)
                    nc.vector.scalar_tensor_tensor(
                        mem_st[:, cs], spk2_prev[0], 0.0, u2[:],
                        op0=Alu.is_equal, op1=Alu.mult)
                    nc.vector.tensor_scalar(
                        spk_st[:, cs], mem_st[:, cs], 1.0, None, op0=Alu.is_gt)
                    spk2_prev[0] = spk_st[:, cs]
                mem_prev[0] = mem_st[:, (csz_ - 1) * BS:csz_ * BS]
                ow = slice(c0_ * BS, (c0_ + csz_) * BS)
                nc.gpsimd.dma_start(spk_d[:, ow], spk_st[:])
                nc.gpsimd.dma_start(mem_d[:, ow], mem_st[:])

            prev = None
            for ci, (c0_, csz_) in enumerate(chunks):
                nw = csz_ * BS
                if ci + 1 < NCH:
                    issue_xdma(ci + 1)
                cur = curpool.tile([128, HT, nw], f32, tag="cur",
                                   name=f"cur{ci}")
                spk = spkpool.tile([128, HT, nw], f16, tag="spk",
                                   name=f"spk{ci}")
                for h in range(HT):
                    ps = ps1pool.tile([128, nw], f32, tag="p1",
                                      name=f"p1_{ci}_{h}")
                    for j in range(KT2):
                        # rows 2352:2432 are zero-padded on host; full-128
                        # tiles keep walrus codegen on the well-trodden path
                        xa, xb = xw[ci]
                        rhs = xa[:, j, :] if j < 10 else xb[:, j - 10, :]
                        nc.tensor.matmul(
                            ps[:], wm[:, h, j, :], rhs,
                            start=(j == 0), stop=(j == KT2 - 1))
                    nc.scalar.activation(cur[:, h, :], ps[:], Act.Copy)
                if prev is not None:
                    tail(*prev)
                prev = (ci, cur, spk)
            tail(*prev)

    nc.compile()
    return nc


def _prep_inputs(x, W1, W2):
    """Host-side layout + fp16 3-term splits."""
    f32 = np.float32
    f16 = np.float16
    SH = f32(2048.0)

    # x: [T, B, N_IN] -> feature-major [N_IN, T, B]
    xt = np.ascontiguousarray(np.transpose(np.asarray(x, f32), (2, 0, 1)))
    xh = xt.astype(f16)
    xl = ((xt - xh.astype(f32)) * SH).astype(f16)

    xcores = []
    for c in range(NCORES):
        bs = slice(c * BS, (c + 1) * BS)
        xc = np.zeros((KT2 * 128, T * BS), dtype=f16)
        xc[0:N_IN] = xh[:, :, bs].reshape(N_IN, T * BS)
        xc[N_IN:2 * N_IN] = xl[:, :, bs].reshape(N_IN, T * BS)
        xc[2 * N_IN:3 * N_IN] = xc[0:N_IN]
        # [KROWSpad, TB] -> [128, KT2, TB]
        xc = np.ascontiguousarray(
            xc.reshape(KT2, 128, T * BS).transpose(1, 0, 2))
        xcores.append(xc)

    W1T = np.ascontiguousarray(np.asarray(W1, f32).T)   # [784, 1024]
    wh = W1T.astype(f16)
    wl = (W1T - wh.astype(f32)).astype(f16)
    whs = (wh.astype(f32) / SH).astype(f16)
    wcat = np.zeros((KT2 * 128, N_HID), dtype=f16)
    wcat[0:N_IN] = wh
    wcat[N_IN:2 * N_IN] = whs
    wcat[2 * N_IN:3 * N_IN] = wl
    # [KROWSpad, 1024] -> [128, HT, KT2, 128] with free=(h, j, m)
    wm = np.ascontiguousarray(
        wcat.reshape(KT2, 128, HT, 128).transpose(1, 2, 0, 3).reshape(
            128, HT, KT2 * 128))

    W2T = np.ascontiguousarray(np.asarray(W2, f32).T)   # [1024, 10]
    w2h = W2T.astype(f16)
    w2l = (W2T - w2h.astype(f32)).astype(f16)

    def w2_layout(a):
        return np.ascontiguousarray(
            a.reshape(HT, 128, N_OUT).transpose(1, 0, 2).reshape(
                128, HT * N_OUT))

    weights = {"wm": wm, "w2h": w2_layout(w2h), "w2l": w2_layout(w2l)}
    return xcores, weights


def _ensure_ntff_shim():
    """run_bass_kernel_spmd(trace) imports antenv.axon_hooks, absent in some
    images; install a graceful stand-in so tracing degrades instead of
    crashing."""
    try:
        import antenv.axon_hooks  # noqa: F401
        return
    except Exception:
        pass
    import types
    hook = None
    try:
        from trn_agent_boot.trn_boot import _ntff_profile_via_ctypes
        hook = _ntff_profile_via_ctypes("/opt/axon/libaxon_pjrt.so")
    except Exception:
        hook = None
    mod = types.ModuleType("antenv.axon_hooks")
    mod._hook = hook
    mod.get_axon_ntff_profile_hook = lambda: mod._hook
    mod.set_axon_ntff_profile_hook = lambda h: setattr(mod, "_hook", h)
    sys.modules["antenv.axon_hooks"] = mod


def kernel(x, W1, W2, beta1, beta2):
    global LAST_RESULT
    from concourse.bass_utils import run_bass_kernel_spmd

    _ensure_ntff_shim()

    b1 = float(np.clip(np.float32(beta1), 0.0, 1.0))
    b2 = float(np.clip(np.float32(beta2), 0.0, 1.0))

    xcores, weights = _prep_inputs(x, W1, W2)
    nc = _build_bass(b1, b2)

    in_maps = []
    for c in range(NCORES):
        m = {"xm": xcores[c]}
        m.update(weights)
        in_maps.append(m)

    res = run_bass_kernel_spmd(nc, in_maps, core_ids=list(range(NCORES)))
    LAST_RESULT = res

    spk_parts, mem_parts = [], []
    for c in range(NCORES):
        r = res.results[c]
        spk_parts.append(
            r["spk2o"].reshape(N_OUT, T, BS).transpose(1, 2, 0))
        mem_parts.append(
            r["mem2o"].reshape(N_OUT, T, BS).transpose(1, 2, 0))
    spk2 = np.ascontiguousarray(np.concatenate(spk_parts, axis=1))
    mem2 = np.ascontiguousarray(np.concatenate(mem_parts, axis=1))
    return spk2, mem2
